# revision 50
# baseline (speedup 1.0000x reference)
"""HOPE block kernel for 8 Trainium2 NeuronCores — sequence-parallel.

Each core owns a contiguous 256-step segment of the sequence for ALL
batches and ALL heads.  The linear-attention memory M = cumsum_t(mean_b
V K^T) is an associative scan over time: each core computes its local
per-segment delta G_seg = sum_{t in seg} mean_b K_t V_t^T, AllGathers
the 8 deltas (tiny, ~1MB, Shared-output collective overlapped with the
local masked-scan matmuls), and masked-accumulates the cross-core
prefix.  Everything else (layernorms, Q/K/V/O projections, CMS MLPs,
residuals) is local to the core's 1024-token shard — no other
communication.

ln1 scale/bias is folded into Wq/Wk/Wv host-side (plus the 1/B on V);
ln2 scale/bias is folded into the level-0 CMS weights.
"""

import numpy as np
import ml_dtypes

import concourse.bass as bass
import concourse.bacc as bacc
import concourse.mybir as mybir
import concourse.tile as tile
from concourse.bass_utils import run_bass_kernel_spmd
from concourse.masks import make_identity

N_CORES = 8
B, S, DIM = 4, 2048, 512
H, D = 8, 64
HID = 4 * DIM
NLVL = 3
EPS = 1e-5
SEG = S // N_CORES       # 256 steps per core
TSH = B * SEG            # 1024 tokens per shard
NT = TSH // 128          # 8 token tiles per shard
SC1, SC2, ASC = 8.0, 16.0, 8.0
FP32 = mybir.dt.float32
BF16 = mybir.dt.bfloat16
FP8 = mybir.dt.float8e4
DR = mybir.MatmulPerfMode.DoubleRow
AX = mybir.AxisListType.X
ALU = mybir.AluOpType
ACTF = mybir.ActivationFunctionType


def _ln_normalize(nc, pool, xt, out_bf, sq_scratch, eps_tile):
    """out_bf = (xt - mean(xt)) * rsqrt(var(xt)+EPS), per 128-token tile."""
    stats = pool.tile([128, 6], FP32, tag="ln_s")
    nc.vector.bn_stats(out=stats[:], in_=xt[:])
    mv = pool.tile([128, 2], FP32, tag="ln_mv")
    nc.vector.bn_aggr(out=mv[:], in_=stats[:])
    std = pool.tile([128, 1], FP32, tag="ln_d")
    nc.scalar.activation(std[:], mv[:, 1:2], ACTF.Sqrt, bias=eps_tile[:])
    rs = pool.tile([128, 1], FP32, tag="ln_r")
    nc.vector.reciprocal(rs[:], std[:])
    nc.vector.tensor_scalar(
        out=out_bf[:], in0=xt[:], scalar1=mv[:, 0:1], scalar2=rs[:],
        op0=ALU.subtract, op1=ALU.mult,
    )


def build_kernel(qkv_bias=True, b2l_zero=False):
    nc = bacc.Bacc(num_devices=N_CORES)

    x_sh = nc.dram_tensor("x_shard", [TSH, DIM], FP32, kind="ExternalInput")
    wqT = nc.dram_tensor("wqT", [DIM, DIM], BF16, kind="ExternalInput")
    wkT = nc.dram_tensor("wkT", [DIM, DIM], BF16, kind="ExternalInput")
    wvT = nc.dram_tensor("wvT", [DIM, DIM], BF16, kind="ExternalInput")
    if qkv_bias:
        qb = nc.dram_tensor("qb", [128, 4], FP32, kind="ExternalInput")
        kb = nc.dram_tensor("kb", [128, 4], FP32, kind="ExternalInput")
        kb_bc = nc.dram_tensor("kb_bc", [128, DIM], FP32, kind="ExternalInput")
        vb_bc = nc.dram_tensor("vb_bc", [128, DIM], FP32, kind="ExternalInput")
    wo_T = nc.dram_tensor("wo_T", [DIM, DIM], BF16, kind="ExternalInput")
    w1 = nc.dram_tensor("w1", [NLVL, DIM, HID], FP8, kind="ExternalInput")
    w2 = nc.dram_tensor("w2", [NLVL, HID, DIM], FP8, kind="ExternalInput")
    b1 = nc.dram_tensor("b1", [NLVL, 128, HID // 128], FP32, kind="ExternalInput")
    b2a = nc.dram_tensor("b2a", [2, 128, DIM // 128], FP32, kind="ExternalInput")
    if not b2l_zero:
        b2last = nc.dram_tensor("b2last", [128, DIM], FP32, kind="ExternalInput")
    pmask = nc.dram_tensor("pmask", [D, N_CORES], FP32, kind="ExternalInput")
    out_sh = nc.dram_tensor("out_shard", [TSH, DIM], FP32, kind="ExternalOutput")

    with tile.TileContext(nc) as tc:
        with tc.tile_pool(name="dram", bufs=1, space="DRAM") as dram, \
             tc.tile_pool(name="dramsh", bufs=1, space="DRAM") as dramsh, \
             tc.tile_pool(name="const", bufs=1) as cpool, \
             tc.tile_pool(name="lns", bufs=4) as lnp, \
             tc.tile_pool(name="xp", bufs=1) as xpool, \
             tc.tile_pool(name="cmsw", bufs=2) as wts, \
             tc.tile_pool(name="cmsb", bufs=2) as bts:

            rs_in = dram.tile([N_CORES * D, H * D], BF16)
            rs_out = dram.tile([D, H * D], BF16)

            # x first so ln1 starts ASAP (DMA issue on the sync queue is
            # serial, ~1us per instruction — keep x ahead of the weights)
            x_sb = xpool.tile([128, NT, DIM], FP32)
            xv = x_sh[:].rearrange("(t p) d -> p t d", p=128)
            for t in range(NT):
                nc.sync.dma_start(x_sb[:, t], xv[:, t])

            identity = cpool.tile([128, 128], BF16)
            make_identity(nc, identity[:])
            # keep-mask: mask[t, s] = 1.0 if t <= s else 0.0, tiled 4x along s
            mask512 = cpool.tile([128, 4, 128], FP32)
            nc.gpsimd.memset(mask512[:], 1.0)
            for i in range(4):
                nc.gpsimd.affine_select(
                    out=mask512[:, i], in_=mask512[:, i], compare_op=ALU.is_ge,
                    fill=0.0, base=0, pattern=[[1, 128]], channel_multiplier=-1,
                )

            wq_sb = cpool.tile([128, 4, DIM], BF16)
            nc.sync.dma_start(wq_sb[:], wqT[:].rearrange("(a p) m -> p a m", p=128))
            wk_sb = cpool.tile([128, 4, DIM], BF16)
            nc.sync.dma_start(wk_sb[:], wkT[:].rearrange("(a p) m -> p a m", p=128))
            wv_sb = cpool.tile([128, 4, DIM], BF16)
            nc.sync.dma_start(wv_sb[:], wvT[:].rearrange("(a p) m -> p a m", p=128))
            woT_sb = cpool.tile([128, 4, DIM], BF16)
            nc.sync.dma_start(woT_sb[:], wo_T[:].rearrange("(a p) m -> p a m", p=128))
            if qkv_bias:
                qb_sb = cpool.tile([128, 4], FP32, name="qb_sb")
                nc.sync.dma_start(qb_sb[:], qb[:])
                kbv_sb = cpool.tile([128, 4], FP32, name="kbv_sb")
                nc.sync.dma_start(kbv_sb[:], kb[:])
                kbc_sb = cpool.tile([128, DIM], FP32, name="kbc_sb")
                nc.sync.dma_start(kbc_sb[:], kb_bc[:])
                vbc_sb = cpool.tile([128, DIM], FP32, name="vbc_sb")
                nc.sync.dma_start(vbc_sb[:], vb_bc[:])
            else:
                qb_sb = kbv_sb = kbc_sb = vbc_sb = None
            pm_sb = cpool.tile([D, N_CORES], FP32)
            nc.sync.dma_start(pm_sb[:], pmask[:])
            if not b2l_zero:
                b2l_sb = cpool.tile([128, DIM], FP32, name="b2l_sb")
                nc.sync.dma_start(b2l_sb[:], b2last[:])
            else:
                b2l_sb = None
            eps_sb = cpool.tile([128, 1], FP32)
            nc.vector.memset(eps_sb[:], EPS)

            # prefetch CMS level-0 weights early (stream rest during compute)
            w1_sb = [None] * NLVL
            w2_sb = [None] * NLVL
            b1_sb = [None] * NLVL
            b2_sb = [None, None]
            for lvl in range(NLVL):
                w1_sb[lvl] = wts.tile([128, 4, 16, 128], FP8, tag="w1", name=f"w1sb{lvl}")
                nc.sync.dma_start(
                    w1_sb[lvl][:],
                    w1[lvl].rearrange("(a p) (ht q) -> p a ht q", p=128, q=128))
                w2_sb[lvl] = wts.tile([128, 16, 4, 128], FP8, tag="w2", name=f"w2sb{lvl}")
                nc.sync.dma_start(
                    w2_sb[lvl][:],
                    w2[lvl].rearrange("(ht p) (a q) -> p ht a q", p=128, q=128))
                b1_sb[lvl] = bts.tile([128, HID // 128], FP32, tag="b1", name=f"b1sb{lvl}")
                nc.sync.dma_start(b1_sb[lvl][:], b1[lvl])
                if lvl < 2:
                    b2_sb[lvl] = bts.tile([128, 4], FP32, tag="b2", name=f"b2sb{lvl}")
                    nc.sync.dma_start(b2_sb[lvl][:], b2a[lvl])

            # persistent buffers (entered early so pool stack pops LIFO)
            h2nT_p = tc.tile_pool(name="h2nT", bufs=1)
            h2_p = tc.tile_pool(name="h2", bufs=1)
            yT_p = tc.tile_pool(name="yT", bufs=1)
            h2nT = h2nT_p.__enter__().tile([128, 4, TSH], FP8)
            h2_sb = h2_p.__enter__().tile([128, NT, DIM], FP32)
            yT = yT_p.__enter__().tile([128, 4, 2, 512], BF16)

            # ---- stage 1: ln1 + transpose to feature-major ----
            hT_p = tc.tile_pool(name="hT", bufs=1)
            hnT = hT_p.__enter__().tile([128, 4, TSH], BF16)
            with tc.tile_pool(name="s1w", bufs=3) as s1w, \
                 tc.tile_pool(name="s1p", bufs=2, space="PSUM") as s1p:
                for t in range(NT):
                    hn = s1w.tile([128, DIM], BF16, tag="hn")
                    sq = s1w.tile([128, DIM], BF16, tag="sq")
                    _ln_normalize(nc, lnp, x_sb[:, t], hn, sq, eps_sb)
                    for a in range(4):
                        ps = s1p.tile([128, 128], BF16)
                        nc.tensor.transpose(ps[:], hn[:, a * 128:(a + 1) * 128], identity[:])
                        nc.vector.tensor_copy(hnT[:, a, t * 128:(t + 1) * 128], ps[:])

            # ---- stage 2: projections + G deltas + AllGather ----
            qkv_p = tc.tile_pool(name="qkv", bufs=1)
            qkvp = qkv_p.__enter__()
            # QT/KT/yT columns in (lc, b, s) order so every scan matmul
            # reads a contiguous 512-wide block; K_td/V_td token-major
            QT = qkvp.tile([128, 4, 2, 512], BF16)
            KT = qkvp.tile([128, 4, 2, 512], BF16)
            K_td = qkvp.tile([128, NT, DIM], BF16)
            V_td = qkvp.tile([128, NT, DIM], BF16)
            Gd_sb = qkvp.tile([D, 2, H * D], FP32)
            # [p=(h%2)*64+dk, lc, r=h//2, dv] so lhsT base matches Q operand base
            Gpre16 = qkvp.tile([128, 2, 4, D], BF16)

            with tc.tile_pool(name="s2p", bufs=2, space="PSUM") as s2p, \
                 tc.tile_pool(name="s2pg", bufs=1, space="PSUM") as s2pg, \
                 tc.tile_pool(name="s2gi", bufs=2) as s2gi, \
                 tc.tile_pool(name="s2w", bufs=1) as s2w:
                # token-major K, V (needed for G deltas — do these first,
                # even-parity chunk tiles first so lc=0 deltas start early)
                def g_mms(lc, b, pgs):
                    # head pairs (2r, 2r+1) run concurrently on disjoint
                    # column groups (out base partitions 0 / 64)
                    for r in range(4):
                        he = slice(2 * r * D, 2 * r * D + D)
                        ho = slice((2 * r + 1) * D, (2 * r + 1) * D + D)
                        nc.tensor.matmul(pgs[r][0:D], K_td[:, 2 * b + lc, he],
                                         V_td[:, 2 * b + lc, he],
                                         start=(b == 0), stop=(b == B - 1))
                        nc.tensor.matmul(pgs[r][D:128], K_td[:, 2 * b + lc, ho],
                                         V_td[:, 2 * b + lc, ho],
                                         start=(b == 0), stop=(b == B - 1))

                for lc in range(2):
                    # G-delta accumulation software-pipelined one tile behind
                    # the K/V projections, so the collective launches early
                    pgs = [s2pg.tile([128, D], FP32, tag=f"g{r}", name=f"pg{r}")
                           for r in range(4)]
                    for bi in range(B):
                        t = 2 * bi + lc
                        pk = s2p.tile([128, DIM], FP32, tag="kv", name="pk")
                        for a in range(4):
                            nc.tensor.matmul(pk[:], hnT[:, a, t * 128:(t + 1) * 128],
                                             wk_sb[:, a], start=(a == 0), stop=(a == 3))
                        if qkv_bias:
                            nc.vector.tensor_tensor(K_td[:, t], pk[:], kbc_sb[:],
                                                    ALU.add)
                        else:
                            nc.scalar.activation(K_td[:, t], pk[:], ACTF.Copy)
                        pv = s2p.tile([128, DIM], FP32, tag="kv", name="pv")
                        for a in range(4):
                            nc.tensor.matmul(pv[:], hnT[:, a, t * 128:(t + 1) * 128],
                                             wv_sb[:, a], start=(a == 0), stop=(a == 3))
                        if qkv_bias:
                            nc.vector.tensor_tensor(V_td[:, t], pv[:], vbc_sb[:],
                                                    ALU.add)
                        else:
                            nc.vector.tensor_copy(V_td[:, t], pv[:])
                        if bi > 0:
                            g_mms(lc, bi - 1, pgs)
                    g_mms(lc, B - 1, pgs)
                    for r in range(4):
                        he = slice(2 * r * D, 2 * r * D + D)
                        ho = slice((2 * r + 1) * D, (2 * r + 1) * D + D)
                        nc.vector.tensor_copy(Gd_sb[:, lc, he], pgs[r][0:D])
                        nc.vector.tensor_copy(Gd_sb[:, lc, ho], pgs[r][D:128])
                gseg = s2w.tile([D, H * D], FP32, tag="gseg")
                nc.vector.tensor_tensor(gseg[:], Gd_sb[:, 0], Gd_sb[:, 1], ALU.add)
                # masked-input ReduceScatter computes the exclusive prefix
                # directly: core c' contributes gseg*[j>c'] to slot j, the
                # add-reduce-scatter hands core c slot c = sum_{c'<c} G_c'.
                for j in range(N_CORES):
                    slot = s2gi.tile([D, H * D], BF16, tag="slot")
                    if j % 2 == 0:
                        nc.vector.tensor_scalar(out=slot[:], in0=gseg[:],
                                                scalar1=pm_sb[:, j:j + 1],
                                                scalar2=None, op0=ALU.mult)
                    else:
                        nc.scalar.activation(slot[:], gseg[:], ACTF.Copy,
                                             scale=pm_sb[:, j:j + 1])
                    nc.sync.dma_start(rs_in[j * D:(j + 1) * D, :], slot[:])
                nc.gpsimd.collective_compute(
                    "ReduceScatter", ALU.add,
                    replica_groups=[list(range(N_CORES))],
                    ins=[rs_in.opt()], outs=[rs_out.opt()],
                )

                # feature-major Q, K
                for dst, wsb, bsb in ((QT, wq_sb, qb_sb), (KT, wk_sb, kbv_sb)):
                    dview = dst[:].rearrange("p r lc (b2 s) -> p r b2 lc s", s=128)
                    for r in range(4):
                        pq = [s2p.tile([128, 512], FP32, tag="pq", name=f"pq{n}") for n in range(2)]
                        for a in range(4):
                            for n in range(2):
                                nc.tensor.matmul(
                                    pq[n][:], wsb[:, a, r * 128:(r + 1) * 128],
                                    hnT[:, a, n * 512:(n + 1) * 512],
                                    start=(a == 0), stop=(a == 3))
                        for n in range(2):
                            if qkv_bias:
                                nc.scalar.activation(
                                    dview[:, r, 2 * n:2 * n + 2], pq[n][:],
                                    ACTF.Identity, bias=bsb[:, r:r + 1])
                            else:
                                nc.scalar.activation(
                                    dview[:, r, 2 * n:2 * n + 2], pq[n][:],
                                    ACTF.Copy)



            # ---- stage 3: local masked scan, then late prefix correction ----
            # pass 1 has no dependency on the AllGather, so the in-order PE
            # queue never stalls on it; pass 2 consumes the cross-core prefix
            # ~100us after the collective was issued.
            with tc.tile_pool(name="tmp", bufs=6) as tmp_pool, \
                 tc.tile_pool(name="pt3", bufs=2, space="PSUM") as pt3, \
                 tc.tile_pool(name="py3", bufs=2, space="PSUM") as py3:
                for r in range(4):
                    he = slice(2 * r * D, 2 * r * D + D)
                    ho = slice((2 * r + 1) * D, (2 * r + 1) * D + D)
                    for lc in range(2):
                        qe = QT[0:D, r, lc]
                        qo = QT[D:128, r, lc]
                        Tm = []
                        for bp in range(B):
                            bs = slice(bp * 128, (bp + 1) * 128)
                            pte = pt3.tile([128, 512], FP32, tag="pte", name="pte")
                            pto = pt3.tile([128, 512], FP32, tag="pto", name="pto")
                            nc.tensor.matmul(pte[:], KT[0:D, r, lc, bs], qe)
                            nc.tensor.matmul(pto[:], KT[D:128, r, lc, bs], qo)
                            tme = tmp_pool.tile([128, 4, 128], BF16, tag="tme",
                                                name="tme")
                            tmo = tmp_pool.tile([128, 4, 128], BF16, tag="tmo",
                                                name="tmo")
                            nc.vector.tensor_tensor(
                                tme[:], pte[:].rearrange("p (b s) -> p b s", b=4),
                                mask512[:], ALU.mult)
                            nc.vector.tensor_tensor(
                                tmo[:], pto[:].rearrange("p (b s) -> p b s", b=4),
                                mask512[:], ALU.mult)
                            Tm.append((tme, tmo))
                        py = py3.tile([128, 512], FP32)
                        for bp in range(B):
                            nc.tensor.matmul(py[0:D], V_td[:, 2 * bp + lc, he],
                                             Tm[bp][0][:].rearrange("p b s -> p (b s)"),
                                             start=(bp == 0), stop=(bp == B - 1))
                            nc.tensor.matmul(py[D:128], V_td[:, 2 * bp + lc, ho],
                                             Tm[bp][1][:].rearrange("p b s -> p (b s)"),
                                             start=(bp == 0), stop=(bp == B - 1))
                        nc.scalar.activation(yT[:, r, lc], py[:], ACTF.Copy)
                # post-collective chain goes here, textually AFTER pass 1:
                # per-engine queue order follows program order, so nothing
                # in pass 1 ever sits behind a collective-dependent wait.
                # post-collective chain on GpSimd/Scalar: the Vector queue
                # is still draining pass-1 masks when the collective lands
                P = qkvp.tile([D, H * D], BF16, name="pref")
                nc.sync.dma_start(P[:], rs_out[:])
                Ppl = qkvp.tile([D, H * D], BF16, name="ppl")
                nc.vector.tensor_tensor(Ppl[:], P[:], Gd_sb[:, 0], ALU.add)
                for h in range(H):
                    p0, r = (h % 2) * D, h // 2
                    hs = slice(h * D, (h + 1) * D)
                    nc.vector.tensor_copy(Gpre16[p0:p0 + D, 0, r], P[:, hs])
                    nc.vector.tensor_copy(Gpre16[p0:p0 + D, 1, r], Ppl[:, hs])

                for r in range(4):
                    for lc in range(2):
                        pz = py3.tile([128, 512], FP32, tag="pz")
                        nc.tensor.matmul(pz[0:D], Gpre16[0:D, lc, r],
                                         QT[0:D, r, lc])
                        nc.tensor.matmul(pz[D:128], Gpre16[D:128, lc, r],
                                         QT[D:128, r, lc])
                        ydst = yT[:, r, lc]
                        nc.vector.tensor_tensor(ydst, pz[:], ydst, ALU.add)
            qkv_p.__exit__(None, None, None)
            hT_p.__exit__(None, None, None)

            # ---- stage 4: Wo + residual + ln2 + transpose ----
            yTv = yT[:].rearrange("p r lc m -> p r (lc m)")
            with tc.tile_pool(name="s4w", bufs=4) as s4w, \
                 tc.tile_pool(name="s4p", bufs=2, space="PSUM") as s4p, \
                 tc.tile_pool(name="s4pt", bufs=2, space="PSUM") as s4pt:
                # two passes: dense Wo matmuls first, then the ln2 chains
                # pipeline on Vector/Scalar while the transposes run dense
                for t in range(NT):
                    po = s4p.tile([128, DIM], FP32)
                    yc = (t % 2) * 512 + (t // 2) * 128
                    for a in range(4):
                        nc.tensor.matmul(po[:], yTv[:, a, yc:yc + 128],
                                         woT_sb[:, a],
                                         start=(a == 0), stop=(a == 3))
                    nc.vector.tensor_tensor(h2_sb[:, t], po[:], x_sb[:, t], ALU.add)
                for t in range(NT):
                    hn = s4w.tile([128, DIM], BF16, tag="hn2")
                    sq = s4w.tile([128, DIM], BF16, tag="sq2")
                    _ln_normalize(nc, lnp, h2_sb[:, t], hn, sq, eps_sb)
                    for a in range(4):
                        ps = s4pt.tile([128, 128], BF16, name="ps4")
                        nc.tensor.transpose(ps[:], hn[:, a * 128:(a + 1) * 128], identity[:])
                        nc.vector.tensor_copy(h2nT[:, a, t * 128:(t + 1) * 128], ps[:])
            yT_p.__exit__(None, None, None)

            # ---- stage 5: CMS chain ----
            with tc.tile_pool(name="g", bufs=1) as gp, \
                 tc.tile_pool(name="s5o", bufs=2) as s5o, \
                 tc.tile_pool(name="s5p", bufs=4, space="PSUM") as s5p:
                g_sb = gp.tile([128, 16, TSH], FP8)
                cur = h2nT
                for lvl in range(NLVL):
                    # fp8 DoubleRow: each matmul consumes a k-subtile PAIR
                    gsc = 1.0 / SC1 if lvl == 0 else 1.0 / (ASC * SC1)
                    for ht in range(16):
                        ps = [s5p.tile([128, 512], FP32, tag="p1", name=f"p1_{n}") for n in range(2)]
                        for a in (0, 2):
                            for n in range(2):
                                nc.tensor.matmul(ps[n][:], w1_sb[lvl][:, a:a + 2, ht],
                                                 cur[:, a:a + 2, n * 512:(n + 1) * 512],
                                                 start=(a == 0), stop=(a == 2),
                                                 perf_mode=DR)
                        for n in range(2):
                            nc.scalar.activation(
                                g_sb[:, ht, n * 512:(n + 1) * 512], ps[n][:],
                                ACTF.Gelu_apprx_tanh, scale=gsc,
                                bias=b1_sb[lvl][:, ht:ht + 1])
                    if lvl < 2:
                        nxt = s5o.tile([128, 4, TSH], FP8, tag="nxt")
                        for r in range(4):
                            ps = [s5p.tile([128, 512], FP32, tag="p2", name=f"p2_{n}") for n in range(2)]
                            for ht in range(0, 16, 2):
                                for n in range(2):
                                    nc.tensor.matmul(ps[n][:], w2_sb[lvl][:, ht:ht + 2, r],
                                                     g_sb[:, ht:ht + 2, n * 512:(n + 1) * 512],
                                                     start=(ht == 0), stop=(ht == 14),
                                                     perf_mode=DR)
                            for n in range(2):
                                # b2a is pre-scaled by ASC host-side
                                nc.scalar.activation(
                                    nxt[:, r, n * 512:(n + 1) * 512], ps[n][:],
                                    ACTF.Identity, scale=ASC / SC2,
                                    bias=b2_sb[lvl][:, r:r + 1])
                        cur = nxt
                    else:
                        # last level: emit token-major, add b2 + residual, write out
                        w2rv = w2_sb[2][:].rearrange("p ht a q -> p ht (a q)")
                        for t in range(NT):
                            ps = s5p.tile([128, 512], FP32, tag="p1")
                            for ht in range(0, 16, 2):
                                nc.tensor.matmul(
                                    ps[:], g_sb[:, ht:ht + 2, t * 128:(t + 1) * 128],
                                    w2rv[:, ht:ht + 2],
                                    start=(ht == 0), stop=(ht == 14),
                                    perf_mode=DR)
                            tmp = s5o.tile([128, DIM], FP32, tag="fin")
                            nc.scalar.activation(tmp[:], ps[:], ACTF.Copy,
                                                 scale=1.0 / SC2)
                            if not b2l_zero:
                                nc.vector.tensor_tensor(tmp[:], tmp[:], b2l_sb[:],
                                                        ALU.add)
                            nc.vector.tensor_tensor(tmp[:], tmp[:], h2_sb[:, t], ALU.add)
                            nc.sync.dma_start(
                                out_sh[:].rearrange("(t p) d -> p t d", p=128)[:, t],
                                tmp[:])
            h2_p.__exit__(None, None, None)
            h2nT_p.__exit__(None, None, None)

    nc.finalize()
    return nc


_NC_CACHE = {}


def _get_nc(qkv_bias, b2l_zero):
    key = ("nc", qkv_bias, b2l_zero)
    if key not in _NC_CACHE:
        _NC_CACHE[key] = build_kernel(qkv_bias, b2l_zero)
    return _NC_CACHE[key]


def _prepare_in_maps(x, Wq, Wk, Wv, Wo, ln1_w, ln1_b, ln2_w, ln2_b,
                     cms_W1, cms_b1, cms_W2, cms_b2):
    bf = ml_dtypes.bfloat16
    f32 = np.float32
    ln1_w = np.asarray(ln1_w, f32); ln1_b = np.asarray(ln1_b, f32)
    ln2_w = np.asarray(ln2_w, f32); ln2_b = np.asarray(ln2_b, f32)
    Wq = np.asarray(Wq, f32); Wk = np.asarray(Wk, f32); Wv = np.asarray(Wv, f32)
    Wo = np.asarray(Wo, f32)

    # fold ln1 scale into Wq/Wk/Wv columns, ln1 bias into additive biases
    wqT = np.ascontiguousarray((Wq * ln1_w[None, :]).T).astype(bf)
    wkT = np.ascontiguousarray((Wk * ln1_w[None, :]).T).astype(bf)
    wvT = np.ascontiguousarray(((Wv * ln1_w[None, :]) / B).T).astype(bf)
    bq = (Wq @ ln1_b).astype(f32); bk = (Wk @ ln1_b).astype(f32)
    bv = ((Wv @ ln1_b) / B).astype(f32)

    W1 = np.asarray(cms_W1, f32).copy(); b1v = np.asarray(cms_b1, f32).copy()
    W2 = np.asarray(cms_W2, f32); b2v = np.asarray(cms_b2, f32)
    b1v[0] = b1v[0] + ln2_b @ W1[0]
    W1[0] = W1[0] * ln2_w[:, None]

    b1r = np.ascontiguousarray(
        b1v.reshape(NLVL, HID // 128, 128).transpose(0, 2, 1)).astype(f32)
    b2ar = np.ascontiguousarray(
        b2v[:2].reshape(2, DIM // 128, 128).transpose(0, 2, 1)).astype(f32)
    b2last = np.broadcast_to(b2v[2], (128, DIM)).astype(f32).copy()

    f8 = ml_dtypes.float8_e4m3
    qkv_bias = bool(np.any(bq) or np.any(bk) or np.any(bv))
    b2l_zero = not np.any(b2v[2])
    shared = {
        "wqT": wqT, "wkT": wkT, "wvT": wvT,
        "wo_T": np.ascontiguousarray(Wo.T).astype(bf),
        "w1": (W1 * SC1).astype(f8), "w2": (W2 * SC2).astype(f8),
        "b1": b1r, "b2a": b2ar * ASC,
    }
    if not b2l_zero:
        shared["b2last"] = b2last
    if qkv_bias:
        shared.update({
            "qb": np.ascontiguousarray(bq.reshape(4, 128).T),
            "kb": np.ascontiguousarray(bk.reshape(4, 128).T),
            "kb_bc": np.broadcast_to(bk, (128, DIM)).copy(),
            "vb_bc": np.broadcast_to(bv, (128, DIM)).copy(),
        })
    x = np.asarray(x, f32)
    in_maps = []
    for c in range(N_CORES):
        pm = np.zeros((D, N_CORES), f32)
        pm[:, c + 1:] = 1.0
        m = dict(shared)
        m["x_shard"] = np.ascontiguousarray(
            x[:, SEG * c:SEG * (c + 1), :].reshape(TSH, DIM))
        m["pmask"] = pm
        in_maps.append(m)
    return in_maps, qkv_bias, b2l_zero


def kernel(x, Wq, Wk, Wv, Wo, ln1_w, ln1_b, ln2_w, ln2_b,
           cms_W1, cms_b1, cms_W2, cms_b2):
    in_maps, qkv_bias, b2l_zero = _prepare_in_maps(
        x, Wq, Wk, Wv, Wo, ln1_w, ln1_b, ln2_w, ln2_b,
        cms_W1, cms_b1, cms_W2, cms_b2)
    nc = _get_nc(qkv_bias, b2l_zero)
    res = run_bass_kernel_spmd(nc, in_maps, core_ids=list(range(N_CORES)))
    out = np.stack([res.results[c]["out_shard"] for c in range(N_CORES)], axis=0)
    # core c holds tokens [b, SEG*c + s] -> [c, b, s, d] -> [b, c*s, d]
    out = out.reshape(N_CORES, B, SEG, DIM).transpose(1, 0, 2, 3)
    return np.ascontiguousarray(out.reshape(B, S, DIM)).astype(np.float32)
